# revision 16
# baseline (speedup 1.0000x reference)
"""Trainium2 Bass kernel for nn_CustomPenaltyLayer (MinMax-inverse penalty loss).

Contract: kernel(**inputs) takes the FULL inputs (x:(1024,4096,8) f32,
min_:(8,), scale_:(8,)) and returns the FULL output (scalar f32), sharding
x row-wise across 8 NeuronCores internally.

Math (reference):
  x_inv = (x.reshape(-1, 8) - min_) / scale_
  d = x_inv[:, 2]; a = x_inv[:, 3]
  dev_pen   = count(~(0 <= d <= 252))
  act_pen   = count(a < 0) + count(a > 22)
  trans_pen = sum over adjacent pairs of [mod(prev,2)==0 & prev<20] *
              [(cur != prev+1) & (cur != 22)]
  num_act   = count(a != 22);  total = dev+act+trans + |num_act - 58|

Only columns 2 and 3 of x are used, so the host slices them out and ships
2 contiguous column arrays to the device (4 MiB/core instead of 16 MiB/core,
4x less HBM traffic; the device still streams every element it needs).

Device work per core (P=128 partitions, 4096 elems/partition, tiled):
  ScalarE : a3 = x3*rs3 + b3 (affine), h = a3*0.5 + 2^23, r2 = 2h - 2^24
            (magic-number round-to-nearest-even), S0 += sum sign(a3).
  Pool    : t1 = a3 - r2 (signed distance to nearest even int), u = |t1|.
  VectorE : C_ev += count(u < tau)   [rare-event detector, see below]
            C_hi += count(a3 > 22)
            cl = clamp(x2, m2, X252); D += count(cl != x2)  [dev_pen, exact:
            aref2 < 0 iff x2 < m2; aref2 > 252 iff x2 > X252 (host-bisected)]

Exactness strategy: all terms that depend on float-rounding boundary cases
(a == 0, a == 22, a == even integer for the transition term) can only
disagree between the device's (x-m)*rs rounding and the reference's (x-m)/s
rounding when a3 lands within a few ulp of an even integer (0 and 22 are
even). Those elements ALWAYS satisfy |a3 - nearest_even(a3)| < tau=2^-14
(worst-case rounding gap is a few hundred ulp < 2.5e-5 for |a|<=128), so
they land in detector-flagged (tile, partition) cells. The host re-scans
flagged cells with exact reference semantics (np.float32 division) and
replaces those cells' counts; unflagged cells are provably exact as-is.
The transition penalty comes entirely from cond-hits (mod(prev,2)==0),
which only occur in flagged cells, so the host computes it exactly there
(successor element read globally - no separate boundary-pair handling).
"""

import os
import sys

for _p in ("/opt/trn_rl_repo", os.path.expanduser("~/.axon_site/_ro/trn_rl_repo")):
    if os.path.isdir(_p) and _p not in sys.path:
        sys.path.append(_p)

import numpy as np

import concourse.bacc as bacc
import concourse.tile as tile
from concourse import mybir
from concourse.bass_utils import run_bass_kernel_spmd

F32 = mybir.dt.float32
BF16 = mybir.dt.bfloat16
ALU = mybir.AluOpType
ACTF = mybir.ActivationFunctionType

MAGIC = 8388608.0                   # 2^23
TAU = 2.0 ** -14
BATCH, TIMESTEPS, D = 1024, 4096, 8
N_ROWS = BATCH * TIMESTEPS          # 4,194,304
N_CORES = 8
ROWS_PER_CORE = N_ROWS // N_CORES   # 524,288
P = 128                             # SBUF partitions
R_LIST = (768, 1536, 1408, 256, 128)    # elems/partition per tile
assert sum(R_LIST) * P == ROWS_PER_CORE
N_T = len(R_LIST)

_NC_CACHE = {}


def _build_nc():
    nc = bacc.Bacc("TRN2", target_bir_lowering=False, debug=False)

    xs3 = nc.dram_tensor("xs3", [ROWS_PER_CORE], F32, kind="ExternalInput")
    xs2 = nc.dram_tensor("xs2", [ROWS_PER_CORE], F32, kind="ExternalInput")
    consts = nc.dram_tensor("consts", [P, 12], F32, kind="ExternalInput")
    accV_d = nc.dram_tensor("accV", [P, 4 * N_T], F32, kind="ExternalOutput")

    x3_flat = xs3.ap()
    x2_flat = xs2.ap()

    with tile.TileContext(nc) as tc:
        with (
            tc.tile_pool(name="xp", bufs=3) as xp,
            tc.tile_pool(name="ap_", bufs=3) as ap_pool,
            tc.tile_pool(name="pp", bufs=3) as pp,
            tc.tile_pool(name="wp", bufs=3) as wp,
            tc.tile_pool(name="acc", bufs=1) as accp,
        ):
            cn = accp.tile([P, 12], F32, tag="consts")
            nc.sync.dma_start(cn[:], consts.ap())
            # Absorb the consts-DMA wait into one dummy ACT op so the loop's
            # first ACT op waits on the x-tile DMA instead (single wait slot).
            dummy = accp.tile([P, 1], F32, tag="dummy")
            nc.scalar.copy(dummy[:], cn[:, 0:1])
            rs3 = cn[:, 0:1]    # f32(1/scale3)
            bw = cn[:, 1:2]     # -min3*rs3 - 11  (w = a3 - 11)
            half = cn[:, 2:3]   # 0.5
            m05 = cn[:, 3:4]    # 2^23 + 0.5 (odd-grid magic)
            two = cn[:, 4:5]    # 2.0
            nmg = cn[:, 5:6]    # -(2^24 + 1)
            zero = cn[:, 6:7]   # 0.0
            m2 = cn[:, 7:8]     # min2 (dev lower threshold, x2-space)
            x252 = cn[:, 8:9]   # bisected upper threshold (x2-space)

            accV = accp.tile([P, 4 * N_T], F32, tag="accV")   # C_ev,C_out,D_lo,D_hi

            off = 0
            for t, r in enumerate(R_LIST):
                x3_t = xp.tile([P, r], F32, tag="x3")
                nc.sync.dma_start(
                    x3_t[:], x3_flat[off:off + P * r].rearrange("(p r) -> p r", r=r))
                x2_t = xp.tile([P, r], F32, tag="x2")
                nc.sync.dma_start(
                    x2_t[:], x2_flat[off:off + P * r].rearrange("(p r) -> p r", r=r))
                x3t = x3_t[:]
                x2t = x2_t[:]
                off += P * r

                # ScalarE: w = a3 - 11; h2/r2p = magic round of w to the
                # nearest odd integer (even integers of a3); w2 = w^2.
                w = ap_pool.tile([P, r], F32, tag="w")
                nc.scalar.activation(w[:], x3t, ACTF.Identity,
                                     bias=bw, scale=rs3)
                h2 = ap_pool.tile([P, r], F32, tag="h2")
                nc.scalar.activation(h2[:], w[:], ACTF.Identity,
                                     bias=m05, scale=half)
                r2p = ap_pool.tile([P, r], F32, tag="r2p")
                nc.scalar.activation(r2p[:], h2[:], ACTF.Identity,
                                     bias=nmg, scale=two)
                w2 = ap_pool.tile([P, r], F32, tag="w2")
                nc.scalar.activation(w2[:], w[:], ACTF.Square, bias=zero)

                # Pool: t1 = distance of w to nearest odd int; u = t1^2.
                # bf16 is plenty for the tau-window detector (5x margin).
                t1 = pp.tile([P, r], BF16, tag="t1")
                nc.gpsimd.tensor_tensor(t1[:], w[:], r2p[:], ALU.subtract)
                u = pp.tile([P, r], BF16, tag="u")
                nc.gpsimd.tensor_tensor(u[:], t1[:], t1[:], ALU.mult)

                # VectorE: 4 single-op counts, least-dependent first so the
                # engine isn't stalled on the Act->Pool detector chain
                jdlo = wp.tile([P, r], F32, tag="jdlo")
                nc.vector.tensor_scalar(jdlo[:], x2t, m2, None,
                                        ALU.is_lt, ALU.add,
                                        accum_out=accV[:, 4 * t + 2:4 * t + 3])
                jdhi = wp.tile([P, r], F32, tag="jdhi")
                nc.vector.tensor_scalar(jdhi[:], x2t, x252, None,
                                        ALU.is_gt, ALU.add,
                                        accum_out=accV[:, 4 * t + 3:4 * t + 4])
                jout = wp.tile([P, r], F32, tag="jout")
                nc.vector.tensor_scalar(jout[:], w2[:], 121.0, None,
                                        ALU.is_gt, ALU.add,
                                        accum_out=accV[:, 4 * t + 1:4 * t + 2])
                ju = wp.tile([P, r], BF16, tag="ju")
                nc.vector.tensor_scalar(ju[:], u[:], TAU * TAU, None,
                                        ALU.is_lt, ALU.add,
                                        accum_out=accV[:, 4 * t:4 * t + 1])

            nc.sync.dma_start(accV_d.ap(), accV[:])

    nc.compile()
    return nc


def _f32(v):
    return np.float32(v)


def _bisect_upper(m, s, lim):
    """Largest f32 v with f32((v - m)/s) <= lim (monotone in v; exact)."""
    m = _f32(m)
    s = _f32(s)
    lim = _f32(lim)

    def f(v):
        with np.errstate(over="ignore"):
            return _f32((_f32(v) - m) / s)

    hi = np.finfo(np.float32).max
    if f(hi) <= lim:
        return hi
    lo = m                      # f(m) == 0 <= lim
    assert f(lo) <= lim
    lo_b = int(lo.view(np.uint32))
    hi_b = int(hi.view(np.uint32))
    # positive floats: bit pattern order == value order
    while hi_b - lo_b > 1:
        mid_b = (lo_b + hi_b) // 2
        v = np.uint32(mid_b).view(np.float32)
        if f(v) <= lim:
            lo_b = mid_b
        else:
            hi_b = mid_b
    return np.uint32(lo_b).view(np.float32)


def _make_consts(min_, scale_):
    m3 = _f32(min_[3])
    s3 = _f32(scale_[3])
    m2 = _f32(min_[2])
    s2 = _f32(scale_[2])
    rs3 = _f32(1.0) / s3
    b3 = _f32(-np.float64(m3) * np.float64(rs3))
    bw = _f32(np.float64(b3) - 11.0)
    x252 = _bisect_upper(m2, s2, 252.0)
    vals = np.array([rs3, bw, 0.5, MAGIC + 0.5, 2.0, -(2.0 * MAGIC + 1.0),
                     0.0, m2, x252, 0.0, 0.0, 0.0], dtype=np.float32)
    return np.broadcast_to(vals, (P, 12)).copy()


def _run_device(x3col, x2col, min_, scale_, trace=False):
    if "nc" not in _NC_CACHE:
        _NC_CACHE["nc"] = _build_nc()
    nc = _NC_CACHE["nc"]
    consts = _make_consts(min_, scale_)
    in_maps = [
        {"xs3": x3col[c * ROWS_PER_CORE:(c + 1) * ROWS_PER_CORE],
         "xs2": x2col[c * ROWS_PER_CORE:(c + 1) * ROWS_PER_CORE],
         "consts": consts}
        for c in range(N_CORES)
    ]
    return run_bass_kernel_spmd(nc, in_maps, list(range(N_CORES)), trace=trace)


def kernel(x, min_, scale_, _trace=False, _return_bkr=False):
    x = np.asarray(x, dtype=np.float32)
    min_ = np.asarray(min_, dtype=np.float32)
    scale_ = np.asarray(scale_, dtype=np.float32)
    xr = x.reshape(-1, D)
    x3col = np.ascontiguousarray(xr[:, 3])
    x2col = np.ascontiguousarray(xr[:, 2])

    bkr = _run_device(x3col, x2col, min_, scale_, trace=_trace)
    results = bkr.results

    m3 = _f32(min_[3])
    s3 = _f32(scale_[3])

    # Per-cell accumulators, cells indexed (core, tile, partition)
    dev = 0.0
    act_lo = 0.0
    act_hi = 0.0
    eq22 = 0.0
    trans = 0.0
    tile_base = np.cumsum([0] + [P * r for r in R_LIST])[:-1]

    for c in range(N_CORES):
        res = results[c]
        aV = res["accV"].astype(np.float64).reshape(P, N_T, 4)
        C_ev = aV[:, :, 0]
        C_out = aV[:, :, 1]
        dev += aV[:, :, 2].sum() + aV[:, :, 3].sum()

        flagged = C_ev > 0.0                                  # [P, N_T]
        unflag = ~flagged
        act_lo += (C_out * unflag).sum()

        # exact host re-scan of flagged cells with reference semantics
        ps, ts_ = np.nonzero(flagged)
        for p, t in zip(ps, ts_):
            r = R_LIST[t]
            start = c * ROWS_PER_CORE + tile_base[t] + p * r
            xs = x3col[start:start + r]
            aref = ((xs - m3) / s3).astype(np.float32)
            act_lo += float(((aref < 0) | (aref > 22.0)).sum())
            eq22 += float((aref == np.float32(22.0)).sum())
            cond = (np.mod(aref, np.float32(2.0)) == 0.0) & (aref < 20.0)
            for j in np.nonzero(cond)[0]:
                i = start + int(j)
                if i + 1 >= N_ROWS:
                    continue
                an = _f32((_f32(x3col[i + 1]) - m3) / s3)
                ap_ = aref[j]
                if (an != ap_ + np.float32(1.0)) and (an != np.float32(22.0)):
                    trans += 1.0

    numact = float(N_ROWS) - eq22
    act = act_lo + act_hi

    # Reproduce the reference's f32 summation order exactly.
    t1 = np.float32(dev)
    t2 = np.float32(act)
    t3 = np.float32(trans)
    t4 = np.float32(abs(numact - 58.0))
    out = np.array(((t1 + t2) + t3) + t4, dtype=np.float32)
    if _return_bkr:
        return out, bkr
    return out
